# revision 66
# baseline (speedup 1.0000x reference)
"""Trainium2 Bass kernel for nn_BiLinearMHSLayer.

Reference computation (per batch element b):
    t  = x @ fc_w.T + fc_b            [S, E]      (S=1024, IN=768, E=256)
    bl = (t @ bi_w.T).reshape(S,L,E) + bias       (L=12)
    out[i,l,j] = sum_e bl[i,l,e] * t[j,e]         [S, L, S]

Sharding: data-parallel over batch B=8 -> one batch element per NeuronCore.

Per-core dataflow (default _GFORM schedule; contraction dims live on SBUF
partitions).  The score is reassociated as
    out[i,l,j] = sum_e' t[i,e'] * G_l[e',j],  G_l[e',j] = sum_e W_l[e,e'] t[j,e]
which lets bi_w act as a PE stationary operand exactly as it arrives from
DRAM (f = l*256+e on partitions) -- no bi_w transposes -- and makes the
score stationaries tT[gh, i-tile] shared across all 12 l's:
    xT   [IN, S] = PE-transpose of x  (bf16, 48 128x128 tiles)
    tT   [E, S]  = fc_wT.T @ xT  + fc_b          (24 matmuls,  N=512)
    gT   [E, L*S] = biw_sb.T @ tT                (96 matmuls,  N=512)
    out  (per l) = tT.T @ G_l                    (384 matmuls, N=512)

(The bias-over-E term of the reference becomes a j-broadcast under this
reassociation; it is exactly zero per the problem spec, and kernel() falls
back to a direct blT schedule if a caller ever passes a nonzero bias.)

The output is written as fp16 (25MB/core instead of 50MB fp32 -- the
dominant HBM term; the harness-visible result is upcast to fp32 on the
host, costing ~5e-4 relative error against a 2e-2 budget).  Output DMAs go
per 2 l-planes (4KB contiguous per partition) rotated across the SP HWDGE
and Pool SWDGE rings, with per-l stores for the final unit so the drain
tail is short.  PSUM->SBUF evacuation alternates 2:1 between DVE and ACT.
Operands are cast to bf16 (fp32 accumulation in PSUM); |err| vs the fp32
reference is ~4.3e-3 of max|out|.
"""

import json

import ml_dtypes
import numpy as np

import concourse.bass as bass
import concourse.mybir as mybir
import concourse.tile as tile
from concourse.bass_utils import run_bass_kernel_spmd

B, S, IN, E, L = 8, 1024, 768, 256, 12
N_CORES = 8
FP32 = mybir.dt.float32
FP16 = mybir.dt.float16
BF16 = mybir.dt.bfloat16
FP8 = mybir.dt.float8e4
DR = mybir.MatmulPerfMode.DoubleRow
ACT_COPY = mybir.ActivationFunctionType.Copy
ACT_IDENT = mybir.ActivationFunctionType.Identity

# ---------------------------------------------------------------------------
# Workaround: walrus on this image rejects instructions carrying more than one
# embedded sem wait ("Too many sync wait commands", CoreV3GenImpl
# setupSyncWait).  Split excess waits onto EventSemaphore instructions
# inserted immediately before, on the same engine (identical semantics: the
# waits execute, in order, before the instruction).
_WAIT_CAPS = {}
_DEFAULT_WAIT_CAP = 1


def _fix_sync_waits(blob: bytes) -> bytes:
    j = json.loads(blob)
    n = 0
    for f in j.get("functions", []):
        for bb in f.get("blocks", []):
            out = []
            for inst in bb.get("instructions", []):
                si = inst.get("sync_info")
                waits = (si or {}).get("on_wait") or []
                cap = _WAIT_CAPS.get(inst.get("opcode"), _DEFAULT_WAIT_CAP)
                if len(waits) > cap:
                    excess, keep = waits[:len(waits) - cap], waits[len(waits) - cap:]
                    for w in excess:
                        n += 1
                        out.append({
                            "debug": inst.get("debug", 0),
                            "engine": inst["engine"],
                            "ins": [],
                            "name": f"waitsplit-{n}",
                            "opcode": "EventSemaphore",
                            "outs": [],
                            "sync_info": {"on_update": [], "on_wait": [w]},
                        })
                    si["on_wait"] = keep
                out.append(inst)
            bb["instructions"] = out
    return json.dumps(j).encode()


# ---------------------------------------------------------------------------
_DMA_SRC_CONST = False  # debug ablation: output DMAs read a constant tile
_EVAC_MOD = 3           # 1 of every _EVAC_MOD evacuations goes to ACT
_FUSE_LH = False        # True: one 3.1MB DMA per (i-tile, j-half) unit
_DMA_RINGS = 2          # rotate output stores across SP HWDGE / Pool SWDGE
_SCORE_N1024 = False    # one N=1024 moving pass: REJECTED by walrus codegen
                        # (s3d3_mm_num_elements: moving N caps at 512)
_TAIL_DVE = False       # force the last unit's final evacs onto DVE
_DMA_TRANSPOSE = False  # xT/biwT via XBAR DMA-transpose instead of PE
_SCORE_KC_OUTER = True  # kc outer (2 LDWEIGHTS per (l,it)) vs inner (4)
_BLT_FULL = True        # full-s blT per f-tile: halves blT stationary loads
_FP8_SCORE = False      # score stage via fp8e4 DoubleRow, hi+lo split
                        # (out = bl_hi.t_hi + bl_hi.t_lo + bl_lo.t_hi);
                        # K=256 in one DR pass -> 6x256-cycle MMs vs 4x512
_EVAC_BALANCE = False   # greedy 3-way evac balancing (DVE/ACT/Pool) instead
                        # of the fixed 2:1 DVE/ACT rotation
_GFORM = False          # reassociated score: out = tT.T @ G_l with
                        # G_l = biw_sb_l.T @ tT.  biw_sb (e on partitions) is
                        # the stationary directly -> no biwT transposes, and
                        # the score stationary tT[gh, it] is shared across all
                        # l -> 96 score LDWEIGHTS instead of 192.  Requires
                        # bias == 0 (true per spec); kernel() falls back to
                        # the blT path otherwise.
# est. engine copy rates (cols/ns) for the balancer + Pool's head start for
# its SWDGE descriptor-generation duty
_RATE = {"DVE": 0.93, "ACT": 0.47, "POOL": 0.60}
_POOL_PRELOAD_NS = 35000.0
_STT_POOL_PENALTY = 1.43  # gpsimd STT runs at 0.42 eff vs 0.60 for copies


def _emit_consts(nc, const_pool, ident_d):
    # Identity for PE transposes, fed as a host-provided input tensor: the
    # on-chip build (gpsimd memset+affine_select) would sit on the Pool queue
    # in front of the input-load DMA triggers and delay them at kernel start.
    ident = const_pool.tile([128, 128], BF16, tag="ident")
    nc.sync.dma_start(out=ident[:], in_=ident_d[:, :])
    stg_const = None
    if _DMA_SRC_CONST:
        stg_const = const_pool.tile([128, 6 * 512], FP16, tag="stg_const")
        nc.vector.memset(stg_const[:], 1.0)
    return ident, stg_const


def _emit_body(nc, tc, pools, dram, ctr, consts):
    """Emit one full per-core computation."""
    x_d, fcw_d, fcb_d, biw_d, bias_d, out_d = dram
    (const_pool, big_pool, in_pool, psum_mm, stg_pool, dram_pool) = pools
    ident, stg_const = consts

    # balancer state: projected busy-ns per engine
    load = {"DVE": 0.0, "ACT": 0.0, "POOL": _POOL_PRELOAD_NS}

    def _emit_on(eng, dst_ap, src_ap, bias_ap, is_stt=False, stt_in1=None):
        if is_stt:
            e = nc.vector if eng == "DVE" else nc.gpsimd
            e.scalar_tensor_tensor(
                out=dst_ap, in0=src_ap,
                scalar=(bias_ap if bias_ap is not None else 0.0),
                in1=stt_in1, op0=mybir.AluOpType.add,
                op1=mybir.AluOpType.subtract)
        elif eng == "ACT":
            if bias_ap is not None:
                nc.scalar.activation(dst_ap, src_ap, ACT_IDENT, bias=bias_ap)
            else:
                nc.scalar.activation(dst_ap, src_ap, ACT_COPY)
        else:
            e = nc.vector if eng == "DVE" else nc.gpsimd
            if bias_ap is not None:
                e.tensor_scalar_add(dst_ap, src_ap, bias_ap)
            else:
                e.tensor_copy(dst_ap, src_ap)

    def balanced(dst_ap, src_ap, bias_ap=None, is_stt=False, stt_in1=None,
                 engines=("DVE", "ACT", "POOL")):
        cols = dst_ap.free_size()
        best, best_t = None, None
        for eng in engines:
            r = _RATE[eng]
            c = cols / r
            if is_stt and eng == "POOL":
                c *= _STT_POOL_PENALTY
            t = load[eng] + c
            if best_t is None or t < best_t:
                best, best_t, best_c = eng, t, c
        load[best] += best_c
        _emit_on(best, dst_ap, src_ap, bias_ap, is_stt, stt_in1)

    def evac(dst_ap, src_ap, bias_ap=None, force_act=False, force_dve=False):
        """PSUM -> SBUF copy (+ optional per-partition bias add).  Either a
        fixed 2:1 DVE/ACT rotation or greedy 3-way balancing."""
        if _EVAC_BALANCE and not (force_act or force_dve):
            balanced(dst_ap, src_ap, bias_ap)
            return
        c = ctr[0]
        ctr[0] += 1
        if force_dve or (not force_act and c % _EVAC_MOD != _EVAC_MOD - 1):
            if bias_ap is not None:
                nc.vector.tensor_scalar_add(dst_ap, src_ap, bias_ap)
            else:
                nc.vector.tensor_copy(dst_ap, src_ap)
        elif bias_ap is not None:
            # Copy doesn't accept an AP bias; Identity does.
            nc.scalar.activation(dst_ap, src_ap, ACT_IDENT, bias=bias_ap)
        else:
            nc.scalar.activation(dst_ap, src_ap, ACT_COPY)

    # ---- persistent SBUF tensors -------------------------------------------
    x_sb = in_pool.tile([128, 8 * 768], BF16, tag="x_sb")       # [s%128, (s/128, i)]
    fcw_sb = in_pool.tile([128, 2 * 768], BF16, tag="fcw_sb")   # [e%128, (e/128, i)]
    fcb_sb = const_pool.tile([128, 2], FP32, tag="fcb_sb")      # col ec: fc_b[ec*128+p]
    bias_sb = const_pool.tile([128, 2], FP32, tag="bias_sb")
    xT = big_pool.tile([128, 6 * 1024], BF16, tag="xT")         # [i%128, (i/128, s)]
    fcwT = big_pool.tile([128, 6 * 256], BF16, tag="fcwT")      # [i%128, (i/128, e)]
    if _GFORM:
        biwT = None
    else:
        biwT = big_pool.tile([128, 2 * 3072], BF16, tag="biwT")  # [e%128, (e/128, f)]
    tT = big_pool.tile([128, 2 * 1024], BF16, tag="tT")         # [e%128, (e/128, s)]
    biw_sb = in_pool.tile([128, 24 * 256], BF16, tag="biw_sb")  # [f%128, (f/128, e)]
    if _FP8_SCORE:
        # hi/lo fp8 split of blT / tT for DoubleRow score matmuls
        bl8h = big_pool.tile([128, 24 * 1024], FP8, tag="bl8h")
        bl8l = big_pool.tile([128, 24 * 1024], FP8, tag="bl8l")
        tT8h = big_pool.tile([128, 2 * 1024], FP8, tag="tT8h")
        tT8l = big_pool.tile([128, 2 * 1024], FP8, tag="tT8l")
        blT = None
    elif _GFORM:
        # G_l[e', j] = sum_e biw[l*256+e, e'] t[j, e]; col (l*2+gh)*1024 + j
        # holds G_l[gh*128 + p, j]
        gT = big_pool.tile([128, 24 * 1024], BF16, tag="gT")
        blT = None
    else:
        blT = big_pool.tile([128, 24 * 1024], BF16, tag="blT")  # [f%128, (f/128, s)]

    # ---- input loads (SWDGE cast-DMA fp32->bf16, merged) -------------------
    # Order = startup critical path on the single SWDGE ring: fc_w (gates the
    # first PE transposes), x half 0 (gates xT/tT), bi_w f-tiles 0-11 (gates
    # the first biwT/blT half), x half 1, bi_w f-tiles 12-23.
    x_src = x_d.rearrange("(n p) i -> p n i", p=128)            # [128, 8, 768]
    x_dst = x_sb[:].rearrange("p (n i) -> p n i", n=8)
    biw_src = biw_d.rearrange("(n p) e -> p n e", p=128)        # [128, 24, 256]
    biw_dst = biw_sb[:].rearrange("p (n e) -> p n e", n=24)
    nc.gpsimd.dma_start(
        out=fcw_sb[:].rearrange("p (n i) -> p n i", n=2),
        in_=fcw_d.rearrange("(n p) i -> p n i", p=128))
    nc.gpsimd.dma_start(out=x_dst[:, 0:4, :], in_=x_src[:, 0:4, :])
    if _GFORM:
        # bi_w isn't consumed until emit_G (~12us in); x half 1 gates tT(1)
        nc.gpsimd.dma_start(out=x_dst[:, 4:8, :], in_=x_src[:, 4:8, :])
        nc.gpsimd.dma_start(out=biw_dst[:, 0:12, :], in_=biw_src[:, 0:12, :])
        nc.gpsimd.dma_start(out=biw_dst[:, 12:24, :], in_=biw_src[:, 12:24, :])
    else:
        nc.gpsimd.dma_start(out=biw_dst[:, 0:12, :], in_=biw_src[:, 0:12, :])
        nc.gpsimd.dma_start(out=x_dst[:, 4:8, :], in_=x_src[:, 4:8, :])
        nc.gpsimd.dma_start(out=biw_dst[:, 12:24, :], in_=biw_src[:, 12:24, :])
    for c in range(2):
        nc.sync.dma_start(out=fcb_sb[:, c:c + 1], in_=fcb_d[c * 128:(c + 1) * 128, :])
        nc.sync.dma_start(out=bias_sb[:, c:c + 1], in_=bias_d[c * 128:(c + 1) * 128, :])

    # ---- building blocks ----------------------------------------------------
    def pe_transpose_group(dst_ap, srcs):
        """Transpose len(srcs) 128x128 blocks into one PSUM bank, evacuate
        with a single wide copy. dst_ap free size must be len(srcs)*128 and
        column-ordered to match srcs."""
        p = psum_mm.tile([128, 512], BF16, tag="pmm")
        for g, src in enumerate(srcs):
            nc.tensor.transpose(p[:, g * 128:(g + 1) * 128], src, ident[:])
        evac(dst_ap, p[:, 0:len(srcs) * 128])

    def emit_fcwT():
        if _DMA_TRANSPOSE:
            # out[p, ic, c] = fcw_sb[c, n*768 + ic*128 + p] = fc_w[n*128+c,
            # ic*128+p] -> fcwT[:, ic*256 + n*128 + c] via strided 3D out
            fcwT_3d = fcwT[:].rearrange("p (ic e) -> p ic e", ic=6)
            for n in range(2):
                nc.sync.dma_start_transpose(
                    out=fcwT_3d[:, :, n * 128:(n + 1) * 128],
                    in_=fcw_sb[:, n * 768:(n + 1) * 768])
            return
        for ic0 in range(0, 6, 2):
            pe_transpose_group(
                fcwT[:, ic0 * 256:(ic0 + 2) * 256],
                [fcw_sb[:, n * 768 + ic * 128:n * 768 + (ic + 1) * 128]
                 for ic in (ic0, ic0 + 1) for n in (0, 1)])

    def emit_xT(nh):
        if _DMA_TRANSPOSE:
            # XBAR transpose per s-tile: out[p, ic, c] = x_sb[c, n*768+ic*128+p]
            xT_3d = xT[:].rearrange("p (ic s) -> p ic s", ic=6)
            for n in range(nh * 4, nh * 4 + 4):
                nc.sync.dma_start_transpose(
                    out=xT_3d[:, :, n * 128:(n + 1) * 128],
                    in_=x_sb[:, n * 768:(n + 1) * 768])
            return
        # xT columns ic*1024 + n*128 for the 4 s-tiles n of half nh
        for ic in range(6):
            pe_transpose_group(
                xT[:, ic * 1024 + nh * 512:ic * 1024 + (nh + 1) * 512],
                [x_sb[:, n * 768 + ic * 128:n * 768 + (ic + 1) * 128]
                 for n in range(nh * 4, nh * 4 + 4)])

    def emit_biwT(fts=range(24)):
        if _DMA_TRANSPOSE:
            biwT_3d = biwT[:].rearrange("p (kc f) -> p kc f", kc=2)
            for ft in fts:
                nc.sync.dma_start_transpose(
                    out=biwT_3d[:, :, ft * 128:(ft + 1) * 128],
                    in_=biw_sb[:, ft * 256:(ft + 1) * 256])
            return
        # biwT columns kc*3072 + ft*128; group 4 consecutive ft per bank
        for kc in range(2):
            for ft0 in range(fts.start, fts.stop, 4):
                pe_transpose_group(
                    biwT[:, kc * 3072 + ft0 * 128:kc * 3072 + (ft0 + 4) * 128],
                    [biw_sb[:, ft * 256 + kc * 128:ft * 256 + (kc + 1) * 128]
                     for ft in range(ft0, ft0 + 4)])

    def emit_tT_full():
        # full-s tT per e-tile: each fcwT stationary feeds both s-halves
        # (12 stationary loads instead of 24)
        for ec in range(2):
            p = psum_mm.tile([128, 1024], FP32, tag="pmm")
            for ic in range(6):
                for sh in range(2):
                    nc.tensor.matmul(
                        p[:, sh * 512:(sh + 1) * 512],
                        fcwT[:, ic * 256 + ec * 128:ic * 256 + (ec + 1) * 128],
                        xT[:, ic * 1024 + sh * 512:ic * 1024 + (sh + 1) * 512],
                        start=(ic == 0), stop=(ic == 5))
            sl = slice(ec * 1024, (ec + 1) * 1024)
            evac(tT[:, sl], p[:], bias_ap=fcb_sb[:, ec:ec + 1])
            if _FP8_SCORE:
                nc.gpsimd.tensor_copy(tT8h[:, sl], tT[:, sl])
                nc.vector.scalar_tensor_tensor(
                    out=tT8l[:, sl], in0=tT[:, sl], scalar=0.0,
                    in1=tT8h[:, sl], op0=mybir.AluOpType.add,
                    op1=mybir.AluOpType.subtract)

    def emit_tT(ns):
        for ec in range(2):
            p = psum_mm.tile([128, 512], FP32, tag="pmm")
            for ic in range(6):
                nc.tensor.matmul(
                    p[:],
                    fcwT[:, ic * 256 + ec * 128:ic * 256 + (ec + 1) * 128],
                    xT[:, ic * 1024 + ns * 512:ic * 1024 + (ns + 1) * 512],
                    start=(ic == 0), stop=(ic == 5))
            sl = slice(ec * 1024 + ns * 512, ec * 1024 + (ns + 1) * 512)
            evac(tT[:, sl], p[:], bias_ap=fcb_sb[:, ec:ec + 1])
            if _FP8_SCORE:
                # fp8 hi = quant(t); lo = quant(t - hi); both read the bf16
                # tT just written (hi as plain cast-copy, lo as STT t - hi).
                if _EVAC_BALANCE:
                    balanced(tT8h[:, sl], tT[:, sl])
                    balanced(tT8l[:, sl], tT[:, sl], bias_ap=None,
                             is_stt=True, stt_in1=tT8h[:, sl],
                             engines=("DVE", "POOL"))
                else:
                    nc.gpsimd.tensor_copy(tT8h[:, sl], tT[:, sl])
                    nc.vector.scalar_tensor_tensor(
                        out=tT8l[:, sl], in0=tT[:, sl], scalar=0.0,
                        in1=tT8h[:, sl], op0=mybir.AluOpType.add,
                        op1=mybir.AluOpType.subtract)

    def emit_blT_full(fts=range(24)):
        # full-s blT per f-tile: one [128,1024] psum, kc outer so each
        # stationary biwT tile feeds BOTH s-halves (2 stationary loads per
        # f-tile instead of 4; stationary changes cost ~125ns serialized)
        for ft in fts:
            p = psum_mm.tile([128, 1024], FP32, tag="pmm")
            for kc in range(2):
                for sh in range(2):
                    nc.tensor.matmul(
                        p[:, sh * 512:(sh + 1) * 512],
                        biwT[:, kc * 3072 + ft * 128:kc * 3072 + (ft + 1) * 128],
                        tT[:, kc * 1024 + sh * 512:kc * 1024 + (sh + 1) * 512],
                        start=(kc == 0), stop=(kc == 1))
            sl = slice(ft * 1024, (ft + 1) * 1024)
            bias_ap = bias_sb[:, ft % 2:ft % 2 + 1]
            evac(blT[:, sl], p[:], bias_ap=bias_ap)

    def emit_blT(c0, w, fts=range(24)):
        # one w-wide column sub-block (s in [c0, c0+w)) for f-tiles in fts
        for ft in fts:
            p = psum_mm.tile([128, 512], FP32, tag="pmm")
            for kc in range(2):
                nc.tensor.matmul(
                    p[:, 0:w],
                    biwT[:, kc * 3072 + ft * 128:kc * 3072 + (ft + 1) * 128],
                    tT[:, kc * 1024 + c0:kc * 1024 + c0 + w],
                    start=(kc == 0), stop=(kc == 1))
            sl = slice(ft * 1024 + c0, ft * 1024 + c0 + w)
            bias_ap = bias_sb[:, ft % 2:ft % 2 + 1]
            if _FP8_SCORE:
                # hi = quant(psum + bias); lo = quant((psum + bias) - hi)
                # via STT (DVE/Pool only -- ACT has no tensor-tensor path).
                evac(bl8h[:, sl], p[:, 0:w], bias_ap=bias_ap)
                if _EVAC_BALANCE:
                    balanced(bl8l[:, sl], p[:, 0:w], bias_ap=bias_ap,
                             is_stt=True, stt_in1=bl8h[:, sl],
                             engines=("DVE", "POOL"))
                else:
                    eng = nc.vector if ctr[2] % 2 == 0 else nc.gpsimd
                    ctr[2] += 1
                    eng.scalar_tensor_tensor(
                        out=bl8l[:, sl], in0=p[:, 0:w], scalar=bias_ap,
                        in1=bl8h[:, sl], op0=mybir.AluOpType.add,
                        op1=mybir.AluOpType.subtract)
            else:
                evac(blT[:, sl], p[:, 0:w], bias_ap=bias_ap)

    def out_dma(out_ap, in_ap):
        # Rotate output stores across independent descriptor-generation
        # paths (SP HWDGE and the otherwise-idle Pool SWDGE) so trigger /
        # completion handling of consecutive stores proceeds in parallel.
        # ACT is deliberately excluded: a dma trigger's sem-wait executes
        # in-order on the issuing queue and would stall ACT's evac copies.
        engines = [nc.sync, nc.gpsimd][:max(1, _DMA_RINGS)]
        eng = engines[ctr[1] % len(engines)]
        ctr[1] += 1
        eng.dma_start(out=out_ap, in_=in_ap)

    def emit_G(ls):
        # G_l[e', j] = sum_e W_l[e, e'] t[j, e]: lhsT = biw_sb chunk
        # [e-part, e'-free] used AS LOADED (no transpose), rhs = tT.
        # 2 LDWEIGHTS + 4 N=512 matmuls per (l, gh).
        for l in ls:
            for gh in range(2):
                p = psum_mm.tile([128, 1024], FP32, tag="pmm")
                for kcw in range(2):
                    sta = biw_sb[:, (2 * l + kcw) * 256 + gh * 128:
                                 (2 * l + kcw) * 256 + (gh + 1) * 128]
                    for jh in range(2):
                        nc.tensor.matmul(
                            p[:, jh * 512:(jh + 1) * 512],
                            sta,
                            tT[:, kcw * 1024 + jh * 512:kcw * 1024 + (jh + 1) * 512],
                            start=(kcw == 0), stop=(kcw == 1))
                evac(gT[:, (2 * l + gh) * 1024:(2 * l + gh + 1) * 1024], p[:])

    def emit_score(lh):
        # out[i, l, j] = sum_e' t[i, e'] G_l[e', j].  Stationary tT[gh, it]
        # serves BOTH l's of the lg pair -> 2 LDWEIGHTS per 8 matmuls.
        # One stg/DMA unit = (it, lg): [128 i, 2 l, 1024 j].
        for it in range(8):
            for lg in range(3):
                l0 = lh * 6 + 2 * lg
                last_unit = (lh == 1 and it == 7 and lg == 2)


                stg = stg_pool.tile([128, 2 * 1024], FP16, tag="stg2")
                p0 = psum_mm.tile([128, 1024], FP32, tag="pmm", name="p0")
                p1 = psum_mm.tile([128, 1024], FP32, tag="pmm", name="p1")
                ps = [p0, p1]
                # zigzag gh across consecutive lg units: the boundary
                # stationary repeats back-to-back, so its second LDWEIGHTS
                # is skipped at emission (4 LDW per (it, l-half), not 6)
                gh_order = (0, 1) if lg % 2 == 0 else (1, 0)
                for gi, gh in enumerate(gh_order):
                    sta = tT[:, gh * 1024 + it * 128:gh * 1024 + (it + 1) * 128]
                    for dl in range(2):
                        for jh in range(2):
                            nc.tensor.matmul(
                                ps[dl][:, jh * 512:(jh + 1) * 512],
                                sta,
                                gT[:, (2 * (l0 + dl) + gh) * 1024 + jh * 512:
                                   (2 * (l0 + dl) + gh) * 1024 + (jh + 1) * 512],
                                start=(gi == 0), stop=(gi == 1))
                for dl in range(2):
                    evac(stg[:, dl * 1024:(dl + 1) * 1024], ps[dl][:])
                if last_unit:
                    # split the final store so the drain tail stays short
                    for dl in range(2):
                        out_dma(
                            out_d[it * 128:(it + 1) * 128, l0 + dl:l0 + dl + 1, :],
                            stg[:, dl * 1024:(dl + 1) * 1024]
                            .rearrange("p (l j) -> p l j", l=1))
                else:
                    out_dma(
                        out_d[it * 128:(it + 1) * 128, l0:l0 + 2, :],
                        stg[:].rearrange("p (l j) -> p l j", l=2))

    def emit_wave(its, lhs=(0, 1)):
        # output unit = (i-tile, l-half) x FULL j: [128 i, 6 l, 1024 j].
        # kc is the OUTER loop so both j-halves reuse one stationary blT tile
        # (2 LDWEIGHTS per (l, i-tile) instead of 4).  Output DMA is issued
        # per 2 l's (4KB contiguous per partition, still full DMA rate) so the
        # store stream starts as soon as each chunk is evacuated and the final
        # unit's drain tail is ~1.5us instead of ~9us.
        last_unit = (its[-1], lhs[-1])
        for it in its:
            for lh in lhs:
                stg = stg_pool.tile([128, 6 * 1024], FP16, tag="stg")
                for ll in range(6):
                    l = lh * 6 + ll
                    p = psum_mm.tile([128, 1024], FP32, tag="pmm")
                    if _FP8_SCORE:
                        # 3-term hi/lo DoubleRow: K=256 in one pass per term.
                        # Term-outer order -> 2 DR LDWEIGHTS per (l, i-tile).
                        bl_h = bl8h[:].rearrange("p (ft s) -> p ft s", ft=24)[
                            :, 2 * l:2 * l + 2, it * 128:(it + 1) * 128]
                        bl_l = bl8l[:].rearrange("p (ft s) -> p ft s", ft=24)[
                            :, 2 * l:2 * l + 2, it * 128:(it + 1) * 128]
                        t_h = tT8h[:].rearrange("p (kc s) -> p kc s", kc=2)
                        t_l = tT8l[:].rearrange("p (kc s) -> p kc s", kc=2)
                        terms = ((bl_h, t_h), (bl_h, t_l), (bl_l, t_h))
                        for ti, (sta, mov) in enumerate(terms):
                            for jh in range(2):
                                nc.tensor.matmul(
                                    p[:, jh * 512:(jh + 1) * 512],
                                    sta,
                                    mov[:, :, jh * 512:(jh + 1) * 512],
                                    start=(ti == 0), stop=(ti == 2),
                                    perf_mode=DR)
                        evac(stg[:, ll * 1024:(ll + 1) * 1024], p[:],
                             force_dve=(_TAIL_DVE and (it, lh) == last_unit and ll >= 4))
                        if (it, lh) == last_unit:
                            out_dma(
                                out_d[it * 128:(it + 1) * 128,
                                      lh * 6 + ll:lh * 6 + ll + 1, :],
                                stg[:, ll * 1024:(ll + 1) * 1024]
                                .rearrange("p (l j) -> p l j", l=1))
                        elif ll % 2 == 1:
                            out_dma(
                                out_d[it * 128:(it + 1) * 128,
                                      lh * 6 + ll - 1:lh * 6 + ll + 1, :],
                                stg[:, (ll - 1) * 1024:(ll + 1) * 1024]
                                .rearrange("p (l j) -> p l j", l=2))
                        continue
                    for kc in range(2):
                        ft = 2 * l + kc
                        if _SCORE_N1024:
                            # single N=1024 moving pass (bf16 max); out spans
                            # 2 PSUM banks of the same tile
                            nc.tensor.matmul(
                                p[:, 0:1024],
                                blT[:, ft * 1024 + it * 128:ft * 1024 + (it + 1) * 128],
                                tT[:, kc * 1024:(kc + 1) * 1024],
                                start=(kc == 0), stop=(kc == 1))
                        else:
                            for jh in range(2):
                                if not _SCORE_KC_OUTER:
                                    continue
                                nc.tensor.matmul(
                                    p[:, jh * 512:(jh + 1) * 512],
                                    blT[:, ft * 1024 + it * 128:ft * 1024 + (it + 1) * 128],
                                    tT[:, kc * 1024 + jh * 512:kc * 1024 + (jh + 1) * 512],
                                    start=(kc == 0), stop=(kc == 1))
                    if not _SCORE_N1024 and not _SCORE_KC_OUTER:
                        for jh in range(2):
                            for kc in range(2):
                                ft = 2 * l + kc
                                nc.tensor.matmul(
                                    p[:, jh * 512:(jh + 1) * 512],
                                    blT[:, ft * 1024 + it * 128:ft * 1024 + (it + 1) * 128],
                                    tT[:, kc * 1024 + jh * 512:kc * 1024 + (jh + 1) * 512],
                                    start=(kc == 0), stop=(kc == 1))
                    # DVE is ~2x faster per element than ACT: keep the final
                    # unit's evacs off ACT so the drain tail is short.
                    evac(stg[:, ll * 1024:(ll + 1) * 1024], p[:],
                         force_dve=(_TAIL_DVE and (it, lh) == last_unit and ll >= 4))
                    if (it, lh) == last_unit:
                        # per-l stores so the drain after the last matmul is
                        # one evac + one small DMA, not a 1.5MB unit store
                        out_dma(
                            out_d[it * 128:(it + 1) * 128,
                                  lh * 6 + ll:lh * 6 + ll + 1, :],
                            stg[:, ll * 1024:(ll + 1) * 1024]
                            .rearrange("p (l j) -> p l j", l=1))
                    elif ll % 2 == 1:
                        out_dma(
                            out_d[it * 128:(it + 1) * 128,
                                  lh * 6 + ll - 1:lh * 6 + ll + 1, :],
                            stg[:, (ll - 1) * 1024:(ll + 1) * 1024]
                            .rearrange("p (l j) -> p l j", l=2))

    # ---- schedule -----------------------------------------------------------
    if _GFORM:
        # G needs tT complete (contraction over the full e'), so both x/tT
        # halves come first; G + score are split by l-half so the first
        # output units ship while G_6..11 is still being produced.
        emit_fcwT()
        emit_xT(0)
        emit_tT(0)
        emit_xT(1)
        emit_tT(1)
        emit_G(range(0, 6))
        emit_score(lh=0)
        emit_G(range(6, 12))
        emit_score(lh=1)
        return
    # blT n-block 0 covers i-tiles 0-3, n-block 1 covers 4-7; tT n-block jh
    # is the j-half.  Waves are ordered so the output DMA stream starts as
    # early as possible and never starves.
    # Full-j output units need both tT halves, so both x/tT halves come
    # first; blT + its weight transposes are still split by l-half so the
    # first units (needing only f-tiles 0-11) ship while f-tiles 12-23 are
    # still being produced.  biwT(0-12) sits between the tT halves to match
    # the input-DMA arrival order (x half 0, bi_w half A, x half 1, bi_w B).
    if _BLT_FULL:
        # full-s blT per l-half: waves then cover all 8 i-tiles of that
        # l-half at once (wave lh0 needs only f-tiles 0-11).  tT stays
        # split by s-half: a full-s tT would gate on x half 1 and open a
        # PE gap at startup that outweighs its 12 saved stationary loads
        # (sim: 129.0us vs 127.8us).
        emit_fcwT()
        emit_xT(0)
        emit_tT(0)
        emit_biwT(range(0, 12))
        emit_xT(1)
        emit_tT(1)
        emit_blT_full(range(0, 12))
        emit_wave((0, 1, 2, 3, 4, 5, 6, 7), lhs=(0,))
        emit_biwT(range(12, 24))
        emit_blT_full(range(12, 24))
        emit_wave((0, 1, 2, 3, 4, 5, 6, 7), lhs=(1,))
        return
    emit_fcwT()
    emit_xT(0)
    emit_tT(0)
    emit_biwT(range(0, 12))
    emit_xT(1)
    emit_tT(1)
    emit_blT(0, 512, range(0, 12))
    emit_wave((0, 1, 2, 3), lhs=(0,))
    emit_biwT(range(12, 24))
    emit_blT(0, 512, range(12, 24))
    emit_wave((0, 1, 2, 3), lhs=(1,))
    emit_blT(512, 512)
    emit_wave((4, 5, 6, 7))


def build_nc(unroll: int = 1):
    """Build the Bass program.  unroll>1 repeats the whole body (for timing
    measurements via wall-clock differencing)."""
    nc = bass.Bass(trn_type="TRN2")
    x_d = nc.dram_tensor("x", [S, IN], FP32, kind="ExternalInput")
    fcw_d = nc.dram_tensor("fc_w", [E, IN], FP32, kind="ExternalInput")
    fcb_d = nc.dram_tensor("fc_b", [E, 1], FP32, kind="ExternalInput")
    biw_d = nc.dram_tensor("bi_w", [E * L, E], FP32, kind="ExternalInput")
    bias_d = nc.dram_tensor("bias", [E, 1], FP32, kind="ExternalInput")
    ident_d = nc.dram_tensor("ident", [128, 128], BF16, kind="ExternalInput")
    out_d = nc.dram_tensor("out", [S, L, S], FP16, kind="ExternalOutput")
    dram = (x_d, fcw_d, fcb_d, biw_d, bias_d, out_d)

    with tile.TileContext(nc) as tc:
        with (
            tc.tile_pool(name="const", bufs=1) as const_pool,
            tc.tile_pool(name="big", bufs=1) as big_pool,
            tc.tile_pool(name="inp", bufs=1) as in_pool,
            tc.tile_pool(name="psum_mm", bufs=4, space="PSUM") as psum_mm,
            tc.tile_pool(name="stg", bufs=3) as stg_pool,
            tc.tile_pool(name="dram", bufs=1, space="DRAM") as dram_pool,
        ):
            pools = (const_pool, big_pool, in_pool, psum_mm, stg_pool,
                     dram_pool)
            ctr = [0, 0, 0]
            consts = _emit_consts(nc, const_pool, ident_d)
            for _ in range(unroll):
                _emit_body(nc, tc, pools, dram, ctr, consts)

    blob = _fix_sync_waits(nc.to_json_bytes())
    nc.to_json_bytes = lambda: blob
    return nc


_CACHE = {}


def _get_nc(unroll: int = 1):
    key = (unroll, _GFORM)
    if key not in _CACHE:
        _CACHE[key] = build_nc(unroll)
    return _CACHE[key]


def kernel(input_tensor, fc_w, fc_b, bi_w, bias):
    global _GFORM
    input_tensor = np.ascontiguousarray(np.asarray(input_tensor, dtype=np.float32))
    fc_w = np.ascontiguousarray(np.asarray(fc_w, dtype=np.float32))
    fc_b = np.ascontiguousarray(np.asarray(fc_b, dtype=np.float32)).reshape(E, 1)
    bi_w = np.ascontiguousarray(np.asarray(bi_w, dtype=np.float32))
    bias = np.ascontiguousarray(np.asarray(bias, dtype=np.float32)).reshape(E, 1)
    assert input_tensor.shape == (B, S, IN)

    # Measured head-to-head, the blT schedule is ~5us/body faster on this
    # hardware than the reassociated G-form (kept above for reference), so
    # it is used unconditionally.  It also applies the bias generally.
    _GFORM = False
    nc = _get_nc()
    ident = np.eye(128, dtype=ml_dtypes.bfloat16)
    in_maps = [
        {"x": input_tensor[c], "fc_w": fc_w, "fc_b": fc_b, "bi_w": bi_w,
         "bias": bias, "ident": ident}
        for c in range(N_CORES)
    ]
    res = run_bass_kernel_spmd(nc, in_maps, core_ids=list(range(N_CORES)))
    out = np.stack([res.results[c]["out"] for c in range(N_CORES)], axis=0)
    return out.astype(np.float32)



# revision 68
# speedup vs baseline: 1.2167x; 1.2167x over previous
"""Trainium2 Bass kernel for nn_BiLinearMHSLayer.

Reference computation (per batch element b):
    t  = x @ fc_w.T + fc_b            [S, E]      (S=1024, IN=768, E=256)
    bl = (t @ bi_w.T).reshape(S,L,E) + bias       (L=12)
    out[i,l,j] = sum_e bl[i,l,e] * t[j,e]         [S, L, S]

Sharding: data-parallel over batch B=8 -> one batch element per NeuronCore.

Per-core dataflow (default _GFORM schedule; contraction dims live on SBUF
partitions).  The score is reassociated as
    out[i,l,j] = sum_e' t[i,e'] * G_l[e',j],  G_l[e',j] = sum_e W_l[e,e'] t[j,e]
which lets bi_w act as a PE stationary operand exactly as it arrives from
DRAM (f = l*256+e on partitions) -- no bi_w transposes -- and makes the
score stationaries tT[gh, i-tile] shared across all 12 l's:
    xT   [IN, S] = PE-transpose of x  (bf16, 48 128x128 tiles)
    tT   [E, S]  = fc_wT.T @ xT  + fc_b          (24 matmuls,  N=512)
    gT   [E, L*S] = biw_sb.T @ tT                (96 matmuls,  N=512)
    out  (per l) = tT.T @ G_l                    (384 matmuls, N=512)

(The bias-over-E term of the reference becomes a j-broadcast under this
reassociation; it is exactly zero per the problem spec, and kernel() falls
back to a direct blT schedule if a caller ever passes a nonzero bias.)

The output is written as fp16 (25MB/core instead of 50MB fp32 -- the
dominant HBM term; the harness-visible result is upcast to fp32 on the
host, costing ~5e-4 relative error against a 2e-2 budget).  Output DMAs go
per 2 l-planes (4KB contiguous per partition) rotated across the SP HWDGE
and Pool SWDGE rings, with per-l stores for the final unit so the drain
tail is short.  PSUM->SBUF evacuation alternates 2:1 between DVE and ACT.
Operands are cast to bf16 (fp32 accumulation in PSUM); |err| vs the fp32
reference is ~4.3e-3 of max|out|.
"""

import json

import ml_dtypes
import numpy as np

import concourse.bass as bass
import concourse.mybir as mybir
import concourse.tile as tile
from concourse.bass_utils import run_bass_kernel_spmd

B, S, IN, E, L = 8, 1024, 768, 256, 12
N_CORES = 8
FP32 = mybir.dt.float32
FP16 = mybir.dt.float16
BF16 = mybir.dt.bfloat16
FP8 = mybir.dt.float8e4
DR = mybir.MatmulPerfMode.DoubleRow
ACT_COPY = mybir.ActivationFunctionType.Copy
ACT_IDENT = mybir.ActivationFunctionType.Identity

# ---------------------------------------------------------------------------
# Workaround: walrus on this image rejects instructions carrying more than one
# embedded sem wait ("Too many sync wait commands", CoreV3GenImpl
# setupSyncWait).  Split excess waits onto EventSemaphore instructions
# inserted immediately before, on the same engine (identical semantics: the
# waits execute, in order, before the instruction).
_WAIT_CAPS = {}
_DEFAULT_WAIT_CAP = 1


def _fix_sync_waits(blob: bytes) -> bytes:
    j = json.loads(blob)
    n = 0
    for f in j.get("functions", []):
        for bb in f.get("blocks", []):
            out = []
            for inst in bb.get("instructions", []):
                si = inst.get("sync_info")
                waits = (si or {}).get("on_wait") or []
                cap = _WAIT_CAPS.get(inst.get("opcode"), _DEFAULT_WAIT_CAP)
                if len(waits) > cap:
                    excess, keep = waits[:len(waits) - cap], waits[len(waits) - cap:]
                    for w in excess:
                        n += 1
                        out.append({
                            "debug": inst.get("debug", 0),
                            "engine": inst["engine"],
                            "ins": [],
                            "name": f"waitsplit-{n}",
                            "opcode": "EventSemaphore",
                            "outs": [],
                            "sync_info": {"on_update": [], "on_wait": [w]},
                        })
                    si["on_wait"] = keep
                out.append(inst)
            bb["instructions"] = out
    return json.dumps(j).encode()


# ---------------------------------------------------------------------------
_DMA_SRC_CONST = False  # debug ablation: output DMAs read a constant tile
_EVAC_MOD = 3           # 1 of every _EVAC_MOD evacuations goes to ACT
_FUSE_LH = False        # True: one 3.1MB DMA per (i-tile, j-half) unit
_DMA_RINGS = 2          # rotate output stores across SP HWDGE / Pool SWDGE
_SCORE_N1024 = False    # one N=1024 moving pass: REJECTED by walrus codegen
                        # (s3d3_mm_num_elements: moving N caps at 512)
_TAIL_DVE = False       # force the last unit's final evacs onto DVE
_TAIL_SPLIT = False     # last unit's final evacs split DVE||ACT half-tiles
_DMA_TRANSPOSE = False  # xT/biwT via XBAR DMA-transpose instead of PE
_SCORE_KC_OUTER = True  # kc outer (2 LDWEIGHTS per (l,it)) vs inner (4)
_BLT_FULL = True        # full-s blT per f-tile: halves blT stationary loads
_FP8_SCORE = False      # score stage via fp8e4 DoubleRow, hi+lo split
                        # (out = bl_hi.t_hi + bl_hi.t_lo + bl_lo.t_hi);
                        # K=256 in one DR pass -> 6x256-cycle MMs vs 4x512
_EVAC_BALANCE = False   # greedy 3-way evac balancing (DVE/ACT/Pool) instead
                        # of the fixed 2:1 DVE/ACT rotation
_GFORM = False          # reassociated score: out = tT.T @ G_l with
                        # G_l = biw_sb_l.T @ tT.  biw_sb (e on partitions) is
                        # the stationary directly -> no biwT transposes, and
                        # the score stationary tT[gh, it] is shared across all
                        # l -> 96 score LDWEIGHTS instead of 192.  Requires
                        # bias == 0 (true per spec); kernel() falls back to
                        # the blT path otherwise.
# est. engine copy rates (cols/ns) for the balancer + Pool's head start for
# its SWDGE descriptor-generation duty
_RATE = {"DVE": 0.93, "ACT": 0.47, "POOL": 0.60}
_POOL_PRELOAD_NS = 35000.0
_STT_POOL_PENALTY = 1.43  # gpsimd STT runs at 0.42 eff vs 0.60 for copies


def _emit_consts(nc, const_pool, ident_d):
    # Identity for PE transposes, fed as a host-provided input tensor: the
    # on-chip build (gpsimd memset+affine_select) would sit on the Pool queue
    # in front of the input-load DMA triggers and delay them at kernel start.
    ident = const_pool.tile([128, 128], BF16, tag="ident")
    nc.sync.dma_start(out=ident[:], in_=ident_d[:, :])
    stg_const = None
    if _DMA_SRC_CONST:
        stg_const = const_pool.tile([128, 6 * 512], FP16, tag="stg_const")
        nc.vector.memset(stg_const[:], 1.0)
    return ident, stg_const


def _emit_body(nc, tc, pools, dram, ctr, consts):
    """Emit one full per-core computation."""
    x_d, fcw_d, fcb_d, biw_d, bias_d, out_d = dram
    (const_pool, big_pool, in_pool, psum_mm, stg_pool, dram_pool) = pools
    ident, stg_const = consts

    # balancer state: projected busy-ns per engine
    load = {"DVE": 0.0, "ACT": 0.0, "POOL": _POOL_PRELOAD_NS}

    def _emit_on(eng, dst_ap, src_ap, bias_ap, is_stt=False, stt_in1=None):
        if is_stt:
            e = nc.vector if eng == "DVE" else nc.gpsimd
            e.scalar_tensor_tensor(
                out=dst_ap, in0=src_ap,
                scalar=(bias_ap if bias_ap is not None else 0.0),
                in1=stt_in1, op0=mybir.AluOpType.add,
                op1=mybir.AluOpType.subtract)
        elif eng == "ACT":
            if bias_ap is not None:
                nc.scalar.activation(dst_ap, src_ap, ACT_IDENT, bias=bias_ap)
            else:
                nc.scalar.activation(dst_ap, src_ap, ACT_COPY)
        else:
            e = nc.vector if eng == "DVE" else nc.gpsimd
            if bias_ap is not None:
                e.tensor_scalar_add(dst_ap, src_ap, bias_ap)
            else:
                e.tensor_copy(dst_ap, src_ap)

    def balanced(dst_ap, src_ap, bias_ap=None, is_stt=False, stt_in1=None,
                 engines=("DVE", "ACT", "POOL")):
        cols = dst_ap.free_size()
        best, best_t = None, None
        for eng in engines:
            r = _RATE[eng]
            c = cols / r
            if is_stt and eng == "POOL":
                c *= _STT_POOL_PENALTY
            t = load[eng] + c
            if best_t is None or t < best_t:
                best, best_t, best_c = eng, t, c
        load[best] += best_c
        _emit_on(best, dst_ap, src_ap, bias_ap, is_stt, stt_in1)

    def evac(dst_ap, src_ap, bias_ap=None, force_act=False, force_dve=False):
        """PSUM -> SBUF copy (+ optional per-partition bias add).  Either a
        fixed 2:1 DVE/ACT rotation or greedy 3-way balancing."""
        if _EVAC_BALANCE and not (force_act or force_dve):
            balanced(dst_ap, src_ap, bias_ap)
            return
        c = ctr[0]
        ctr[0] += 1
        if force_dve or (not force_act and c % _EVAC_MOD != _EVAC_MOD - 1):
            if bias_ap is not None:
                nc.vector.tensor_scalar_add(dst_ap, src_ap, bias_ap)
            else:
                nc.vector.tensor_copy(dst_ap, src_ap)
        elif bias_ap is not None:
            # Copy doesn't accept an AP bias; Identity does.
            nc.scalar.activation(dst_ap, src_ap, ACT_IDENT, bias=bias_ap)
        else:
            nc.scalar.activation(dst_ap, src_ap, ACT_COPY)

    # ---- persistent SBUF tensors -------------------------------------------
    x_sb = in_pool.tile([128, 8 * 768], BF16, tag="x_sb")       # [s%128, (s/128, i)]
    fcw_sb = in_pool.tile([128, 2 * 768], BF16, tag="fcw_sb")   # [e%128, (e/128, i)]
    fcb_sb = const_pool.tile([128, 2], FP32, tag="fcb_sb")      # col ec: fc_b[ec*128+p]
    bias_sb = const_pool.tile([128, 2], FP32, tag="bias_sb")
    xT = big_pool.tile([128, 6 * 1024], BF16, tag="xT")         # [i%128, (i/128, s)]
    fcwT = big_pool.tile([128, 6 * 256], BF16, tag="fcwT")      # [i%128, (i/128, e)]
    if _GFORM:
        biwT = None
    else:
        biwT = big_pool.tile([128, 2 * 3072], BF16, tag="biwT")  # [e%128, (e/128, f)]
    tT = big_pool.tile([128, 2 * 1024], BF16, tag="tT")         # [e%128, (e/128, s)]
    biw_sb = in_pool.tile([128, 24 * 256], BF16, tag="biw_sb")  # [f%128, (f/128, e)]
    if _FP8_SCORE:
        # hi/lo fp8 split of blT / tT for DoubleRow score matmuls
        bl8h = big_pool.tile([128, 24 * 1024], FP8, tag="bl8h")
        bl8l = big_pool.tile([128, 24 * 1024], FP8, tag="bl8l")
        tT8h = big_pool.tile([128, 2 * 1024], FP8, tag="tT8h")
        tT8l = big_pool.tile([128, 2 * 1024], FP8, tag="tT8l")
        blT = None
    elif _GFORM:
        # G_l[e', j] = sum_e biw[l*256+e, e'] t[j, e]; col (l*2+gh)*1024 + j
        # holds G_l[gh*128 + p, j]
        gT = big_pool.tile([128, 24 * 1024], BF16, tag="gT")
        blT = None
    else:
        blT = big_pool.tile([128, 24 * 1024], BF16, tag="blT")  # [f%128, (f/128, s)]

    # ---- input loads (SWDGE cast-DMA fp32->bf16, merged) -------------------
    # Order = startup critical path on the single SWDGE ring: fc_w (gates the
    # first PE transposes), x half 0 (gates xT/tT), bi_w f-tiles 0-11 (gates
    # the first biwT/blT half), x half 1, bi_w f-tiles 12-23.
    x_src = x_d.rearrange("(n p) i -> p n i", p=128)            # [128, 8, 768]
    x_dst = x_sb[:].rearrange("p (n i) -> p n i", n=8)
    biw_src = biw_d.rearrange("(n p) e -> p n e", p=128)        # [128, 24, 256]
    biw_dst = biw_sb[:].rearrange("p (n e) -> p n e", n=24)
    nc.gpsimd.dma_start(
        out=fcw_sb[:].rearrange("p (n i) -> p n i", n=2),
        in_=fcw_d.rearrange("(n p) i -> p n i", p=128))
    nc.gpsimd.dma_start(out=x_dst[:, 0:4, :], in_=x_src[:, 0:4, :])
    if _GFORM:
        # bi_w isn't consumed until emit_G (~12us in); x half 1 gates tT(1)
        nc.gpsimd.dma_start(out=x_dst[:, 4:8, :], in_=x_src[:, 4:8, :])
        nc.gpsimd.dma_start(out=biw_dst[:, 0:12, :], in_=biw_src[:, 0:12, :])
        nc.gpsimd.dma_start(out=biw_dst[:, 12:24, :], in_=biw_src[:, 12:24, :])
    else:
        nc.gpsimd.dma_start(out=biw_dst[:, 0:12, :], in_=biw_src[:, 0:12, :])
        nc.gpsimd.dma_start(out=x_dst[:, 4:8, :], in_=x_src[:, 4:8, :])
        nc.gpsimd.dma_start(out=biw_dst[:, 12:24, :], in_=biw_src[:, 12:24, :])
    for c in range(2):
        nc.sync.dma_start(out=fcb_sb[:, c:c + 1], in_=fcb_d[c * 128:(c + 1) * 128, :])
        nc.sync.dma_start(out=bias_sb[:, c:c + 1], in_=bias_d[c * 128:(c + 1) * 128, :])

    # ---- building blocks ----------------------------------------------------
    def pe_transpose_group(dst_ap, srcs):
        """Transpose len(srcs) 128x128 blocks into one PSUM bank, evacuate
        with a single wide copy. dst_ap free size must be len(srcs)*128 and
        column-ordered to match srcs."""
        p = psum_mm.tile([128, 512], BF16, tag="pmm")
        for g, src in enumerate(srcs):
            nc.tensor.transpose(p[:, g * 128:(g + 1) * 128], src, ident[:])
        evac(dst_ap, p[:, 0:len(srcs) * 128])

    def emit_fcwT():
        if _DMA_TRANSPOSE:
            # out[p, ic, c] = fcw_sb[c, n*768 + ic*128 + p] = fc_w[n*128+c,
            # ic*128+p] -> fcwT[:, ic*256 + n*128 + c] via strided 3D out
            fcwT_3d = fcwT[:].rearrange("p (ic e) -> p ic e", ic=6)
            for n in range(2):
                nc.sync.dma_start_transpose(
                    out=fcwT_3d[:, :, n * 128:(n + 1) * 128],
                    in_=fcw_sb[:, n * 768:(n + 1) * 768])
            return
        for ic0 in range(0, 6, 2):
            pe_transpose_group(
                fcwT[:, ic0 * 256:(ic0 + 2) * 256],
                [fcw_sb[:, n * 768 + ic * 128:n * 768 + (ic + 1) * 128]
                 for ic in (ic0, ic0 + 1) for n in (0, 1)])

    def emit_xT(nh):
        if _DMA_TRANSPOSE:
            # XBAR transpose per s-tile: out[p, ic, c] = x_sb[c, n*768+ic*128+p]
            xT_3d = xT[:].rearrange("p (ic s) -> p ic s", ic=6)
            for n in range(nh * 4, nh * 4 + 4):
                nc.sync.dma_start_transpose(
                    out=xT_3d[:, :, n * 128:(n + 1) * 128],
                    in_=x_sb[:, n * 768:(n + 1) * 768])
            return
        # xT columns ic*1024 + n*128 for the 4 s-tiles n of half nh
        for ic in range(6):
            pe_transpose_group(
                xT[:, ic * 1024 + nh * 512:ic * 1024 + (nh + 1) * 512],
                [x_sb[:, n * 768 + ic * 128:n * 768 + (ic + 1) * 128]
                 for n in range(nh * 4, nh * 4 + 4)])

    def emit_biwT(fts=range(24)):
        if _DMA_TRANSPOSE:
            biwT_3d = biwT[:].rearrange("p (kc f) -> p kc f", kc=2)
            for ft in fts:
                nc.sync.dma_start_transpose(
                    out=biwT_3d[:, :, ft * 128:(ft + 1) * 128],
                    in_=biw_sb[:, ft * 256:(ft + 1) * 256])
            return
        # biwT columns kc*3072 + ft*128; group 4 consecutive ft per bank
        for kc in range(2):
            for ft0 in range(fts.start, fts.stop, 4):
                pe_transpose_group(
                    biwT[:, kc * 3072 + ft0 * 128:kc * 3072 + (ft0 + 4) * 128],
                    [biw_sb[:, ft * 256 + kc * 128:ft * 256 + (kc + 1) * 128]
                     for ft in range(ft0, ft0 + 4)])

    def emit_tT_full():
        # full-s tT per e-tile: each fcwT stationary feeds both s-halves
        # (12 stationary loads instead of 24)
        for ec in range(2):
            p = psum_mm.tile([128, 1024], FP32, tag="pmm")
            for ic in range(6):
                for sh in range(2):
                    nc.tensor.matmul(
                        p[:, sh * 512:(sh + 1) * 512],
                        fcwT[:, ic * 256 + ec * 128:ic * 256 + (ec + 1) * 128],
                        xT[:, ic * 1024 + sh * 512:ic * 1024 + (sh + 1) * 512],
                        start=(ic == 0), stop=(ic == 5))
            sl = slice(ec * 1024, (ec + 1) * 1024)
            evac(tT[:, sl], p[:], bias_ap=fcb_sb[:, ec:ec + 1])
            if _FP8_SCORE:
                nc.gpsimd.tensor_copy(tT8h[:, sl], tT[:, sl])
                nc.vector.scalar_tensor_tensor(
                    out=tT8l[:, sl], in0=tT[:, sl], scalar=0.0,
                    in1=tT8h[:, sl], op0=mybir.AluOpType.add,
                    op1=mybir.AluOpType.subtract)

    def emit_tT(ns):
        for ec in range(2):
            p = psum_mm.tile([128, 512], FP32, tag="pmm")
            for ic in range(6):
                nc.tensor.matmul(
                    p[:],
                    fcwT[:, ic * 256 + ec * 128:ic * 256 + (ec + 1) * 128],
                    xT[:, ic * 1024 + ns * 512:ic * 1024 + (ns + 1) * 512],
                    start=(ic == 0), stop=(ic == 5))
            sl = slice(ec * 1024 + ns * 512, ec * 1024 + (ns + 1) * 512)
            evac(tT[:, sl], p[:], bias_ap=fcb_sb[:, ec:ec + 1])
            if _FP8_SCORE:
                # fp8 hi = quant(t); lo = quant(t - hi); both read the bf16
                # tT just written (hi as plain cast-copy, lo as STT t - hi).
                if _EVAC_BALANCE:
                    balanced(tT8h[:, sl], tT[:, sl])
                    balanced(tT8l[:, sl], tT[:, sl], bias_ap=None,
                             is_stt=True, stt_in1=tT8h[:, sl],
                             engines=("DVE", "POOL"))
                else:
                    nc.gpsimd.tensor_copy(tT8h[:, sl], tT[:, sl])
                    nc.vector.scalar_tensor_tensor(
                        out=tT8l[:, sl], in0=tT[:, sl], scalar=0.0,
                        in1=tT8h[:, sl], op0=mybir.AluOpType.add,
                        op1=mybir.AluOpType.subtract)

    def emit_blT_full(fts=range(24)):
        # full-s blT per f-tile: one [128,1024] psum, kc outer so each
        # stationary biwT tile feeds BOTH s-halves (2 stationary loads per
        # f-tile instead of 4; stationary changes cost ~125ns serialized)
        for ft in fts:
            p = psum_mm.tile([128, 1024], FP32, tag="pmm")
            for kc in range(2):
                for sh in range(2):
                    nc.tensor.matmul(
                        p[:, sh * 512:(sh + 1) * 512],
                        biwT[:, kc * 3072 + ft * 128:kc * 3072 + (ft + 1) * 128],
                        tT[:, kc * 1024 + sh * 512:kc * 1024 + (sh + 1) * 512],
                        start=(kc == 0), stop=(kc == 1))
            sl = slice(ft * 1024, (ft + 1) * 1024)
            bias_ap = bias_sb[:, ft % 2:ft % 2 + 1]
            evac(blT[:, sl], p[:], bias_ap=bias_ap)

    def emit_blT(c0, w, fts=range(24)):
        # one w-wide column sub-block (s in [c0, c0+w)) for f-tiles in fts
        for ft in fts:
            p = psum_mm.tile([128, 512], FP32, tag="pmm")
            for kc in range(2):
                nc.tensor.matmul(
                    p[:, 0:w],
                    biwT[:, kc * 3072 + ft * 128:kc * 3072 + (ft + 1) * 128],
                    tT[:, kc * 1024 + c0:kc * 1024 + c0 + w],
                    start=(kc == 0), stop=(kc == 1))
            sl = slice(ft * 1024 + c0, ft * 1024 + c0 + w)
            bias_ap = bias_sb[:, ft % 2:ft % 2 + 1]
            if _FP8_SCORE:
                # hi = quant(psum + bias); lo = quant((psum + bias) - hi)
                # via STT (DVE/Pool only -- ACT has no tensor-tensor path).
                evac(bl8h[:, sl], p[:, 0:w], bias_ap=bias_ap)
                if _EVAC_BALANCE:
                    balanced(bl8l[:, sl], p[:, 0:w], bias_ap=bias_ap,
                             is_stt=True, stt_in1=bl8h[:, sl],
                             engines=("DVE", "POOL"))
                else:
                    eng = nc.vector if ctr[2] % 2 == 0 else nc.gpsimd
                    ctr[2] += 1
                    eng.scalar_tensor_tensor(
                        out=bl8l[:, sl], in0=p[:, 0:w], scalar=bias_ap,
                        in1=bl8h[:, sl], op0=mybir.AluOpType.add,
                        op1=mybir.AluOpType.subtract)
            else:
                evac(blT[:, sl], p[:, 0:w], bias_ap=bias_ap)

    def out_dma(out_ap, in_ap):
        # Rotate output stores across independent descriptor-generation
        # paths (SP HWDGE and the otherwise-idle Pool SWDGE) so trigger /
        # completion handling of consecutive stores proceeds in parallel.
        # ACT is deliberately excluded: a dma trigger's sem-wait executes
        # in-order on the issuing queue and would stall ACT's evac copies.
        engines = [nc.sync, nc.gpsimd][:max(1, _DMA_RINGS)]
        eng = engines[ctr[1] % len(engines)]
        ctr[1] += 1
        eng.dma_start(out=out_ap, in_=in_ap)

    def emit_G(ls):
        # G_l[e', j] = sum_e W_l[e, e'] t[j, e]: lhsT = biw_sb chunk
        # [e-part, e'-free] used AS LOADED (no transpose), rhs = tT.
        # 2 LDWEIGHTS + 4 N=512 matmuls per (l, gh).
        for l in ls:
            for gh in range(2):
                p = psum_mm.tile([128, 1024], FP32, tag="pmm")
                for kcw in range(2):
                    sta = biw_sb[:, (2 * l + kcw) * 256 + gh * 128:
                                 (2 * l + kcw) * 256 + (gh + 1) * 128]
                    for jh in range(2):
                        nc.tensor.matmul(
                            p[:, jh * 512:(jh + 1) * 512],
                            sta,
                            tT[:, kcw * 1024 + jh * 512:kcw * 1024 + (jh + 1) * 512],
                            start=(kcw == 0), stop=(kcw == 1))
                evac(gT[:, (2 * l + gh) * 1024:(2 * l + gh + 1) * 1024], p[:])

    def emit_score(lh):
        # out[i, l, j] = sum_e' t[i, e'] G_l[e', j].  Stationary tT[gh, it]
        # serves BOTH l's of the lg pair -> 2 LDWEIGHTS per 8 matmuls.
        # One stg/DMA unit = (it, lg): [128 i, 2 l, 1024 j].
        for it in range(8):
            for lg in range(3):
                l0 = lh * 6 + 2 * lg
                last_unit = (lh == 1 and it == 7 and lg == 2)


                stg = stg_pool.tile([128, 2 * 1024], FP16, tag="stg2")
                p0 = psum_mm.tile([128, 1024], FP32, tag="pmm", name="p0")
                p1 = psum_mm.tile([128, 1024], FP32, tag="pmm", name="p1")
                ps = [p0, p1]
                # zigzag gh across consecutive lg units: the boundary
                # stationary repeats back-to-back, so its second LDWEIGHTS
                # is skipped at emission (4 LDW per (it, l-half), not 6)
                gh_order = (0, 1) if lg % 2 == 0 else (1, 0)
                for gi, gh in enumerate(gh_order):
                    sta = tT[:, gh * 1024 + it * 128:gh * 1024 + (it + 1) * 128]
                    for dl in range(2):
                        for jh in range(2):
                            nc.tensor.matmul(
                                ps[dl][:, jh * 512:(jh + 1) * 512],
                                sta,
                                gT[:, (2 * (l0 + dl) + gh) * 1024 + jh * 512:
                                   (2 * (l0 + dl) + gh) * 1024 + (jh + 1) * 512],
                                start=(gi == 0), stop=(gi == 1))
                for dl in range(2):
                    evac(stg[:, dl * 1024:(dl + 1) * 1024], ps[dl][:])
                if last_unit:
                    # split the final store so the drain tail stays short
                    for dl in range(2):
                        out_dma(
                            out_d[it * 128:(it + 1) * 128, l0 + dl:l0 + dl + 1, :],
                            stg[:, dl * 1024:(dl + 1) * 1024]
                            .rearrange("p (l j) -> p l j", l=1))
                else:
                    out_dma(
                        out_d[it * 128:(it + 1) * 128, l0:l0 + 2, :],
                        stg[:].rearrange("p (l j) -> p l j", l=2))

    def emit_wave(its, lhs=(0, 1)):
        # output unit = (i-tile, l-half) x FULL j: [128 i, 6 l, 1024 j].
        # kc is the OUTER loop so both j-halves reuse one stationary blT tile
        # (2 LDWEIGHTS per (l, i-tile) instead of 4).  Output DMA is issued
        # per 2 l's (4KB contiguous per partition, still full DMA rate) so the
        # store stream starts as soon as each chunk is evacuated and the final
        # unit's drain tail is ~1.5us instead of ~9us.
        last_unit = (its[-1], lhs[-1])
        for it in its:
            for lh in lhs:
                stg = stg_pool.tile([128, 6 * 1024], FP16, tag="stg")
                for ll in range(6):
                    l = lh * 6 + ll
                    p = psum_mm.tile([128, 1024], FP32, tag="pmm")
                    if _FP8_SCORE:
                        # 3-term hi/lo DoubleRow: K=256 in one pass per term.
                        # Term-outer order -> 2 DR LDWEIGHTS per (l, i-tile).
                        bl_h = bl8h[:].rearrange("p (ft s) -> p ft s", ft=24)[
                            :, 2 * l:2 * l + 2, it * 128:(it + 1) * 128]
                        bl_l = bl8l[:].rearrange("p (ft s) -> p ft s", ft=24)[
                            :, 2 * l:2 * l + 2, it * 128:(it + 1) * 128]
                        t_h = tT8h[:].rearrange("p (kc s) -> p kc s", kc=2)
                        t_l = tT8l[:].rearrange("p (kc s) -> p kc s", kc=2)
                        terms = ((bl_h, t_h), (bl_h, t_l), (bl_l, t_h))
                        for ti, (sta, mov) in enumerate(terms):
                            for jh in range(2):
                                nc.tensor.matmul(
                                    p[:, jh * 512:(jh + 1) * 512],
                                    sta,
                                    mov[:, :, jh * 512:(jh + 1) * 512],
                                    start=(ti == 0), stop=(ti == 2),
                                    perf_mode=DR)
                        evac(stg[:, ll * 1024:(ll + 1) * 1024], p[:],
                             force_dve=(_TAIL_DVE and (it, lh) == last_unit and ll >= 4))
                        if (it, lh) == last_unit:
                            out_dma(
                                out_d[it * 128:(it + 1) * 128,
                                      lh * 6 + ll:lh * 6 + ll + 1, :],
                                stg[:, ll * 1024:(ll + 1) * 1024]
                                .rearrange("p (l j) -> p l j", l=1))
                        elif ll % 2 == 1:
                            out_dma(
                                out_d[it * 128:(it + 1) * 128,
                                      lh * 6 + ll - 1:lh * 6 + ll + 1, :],
                                stg[:, (ll - 1) * 1024:(ll + 1) * 1024]
                                .rearrange("p (l j) -> p l j", l=2))
                        continue
                    for kc in range(2):
                        ft = 2 * l + kc
                        if _SCORE_N1024:
                            # single N=1024 moving pass (bf16 max); out spans
                            # 2 PSUM banks of the same tile
                            nc.tensor.matmul(
                                p[:, 0:1024],
                                blT[:, ft * 1024 + it * 128:ft * 1024 + (it + 1) * 128],
                                tT[:, kc * 1024:(kc + 1) * 1024],
                                start=(kc == 0), stop=(kc == 1))
                        else:
                            for jh in range(2):
                                if not _SCORE_KC_OUTER:
                                    continue
                                nc.tensor.matmul(
                                    p[:, jh * 512:(jh + 1) * 512],
                                    blT[:, ft * 1024 + it * 128:ft * 1024 + (it + 1) * 128],
                                    tT[:, kc * 1024 + jh * 512:kc * 1024 + (jh + 1) * 512],
                                    start=(kc == 0), stop=(kc == 1))
                    if not _SCORE_N1024 and not _SCORE_KC_OUTER:
                        for jh in range(2):
                            for kc in range(2):
                                ft = 2 * l + kc
                                nc.tensor.matmul(
                                    p[:, jh * 512:(jh + 1) * 512],
                                    blT[:, ft * 1024 + it * 128:ft * 1024 + (it + 1) * 128],
                                    tT[:, kc * 1024 + jh * 512:kc * 1024 + (jh + 1) * 512],
                                    start=(kc == 0), stop=(kc == 1))
                    # DVE is ~2x faster per element than ACT: keep the final
                    # unit's evacs off ACT so the drain tail is short.
                    if _TAIL_SPLIT and (it, lh) == last_unit and ll >= 4:
                        # halve the critical-path evac latency of the final
                        # chunks: DVE and ACT each copy one j-half
                        nc.vector.tensor_copy(
                            stg[:, ll * 1024:ll * 1024 + 512], p[:, 0:512])
                        nc.scalar.activation(
                            stg[:, ll * 1024 + 512:(ll + 1) * 1024],
                            p[:, 512:1024], ACT_COPY)
                        ctr[0] += 1
                    else:
                        evac(stg[:, ll * 1024:(ll + 1) * 1024], p[:],
                             force_dve=(_TAIL_DVE and (it, lh) == last_unit and ll >= 4))
                    if (it, lh) == last_unit:
                        # per-l stores so the drain after the last matmul is
                        # one evac + one small DMA, not a 1.5MB unit store
                        out_dma(
                            out_d[it * 128:(it + 1) * 128,
                                  lh * 6 + ll:lh * 6 + ll + 1, :],
                            stg[:, ll * 1024:(ll + 1) * 1024]
                            .rearrange("p (l j) -> p l j", l=1))
                    elif ll % 2 == 1:
                        out_dma(
                            out_d[it * 128:(it + 1) * 128,
                                  lh * 6 + ll - 1:lh * 6 + ll + 1, :],
                            stg[:, (ll - 1) * 1024:(ll + 1) * 1024]
                            .rearrange("p (l j) -> p l j", l=2))

    # ---- schedule -----------------------------------------------------------
    if _GFORM:
        # G needs tT complete (contraction over the full e'), so both x/tT
        # halves come first; G + score are split by l-half so the first
        # output units ship while G_6..11 is still being produced.
        emit_fcwT()
        emit_xT(0)
        emit_tT(0)
        emit_xT(1)
        emit_tT(1)
        emit_G(range(0, 6))
        emit_score(lh=0)
        emit_G(range(6, 12))
        emit_score(lh=1)
        return
    # blT n-block 0 covers i-tiles 0-3, n-block 1 covers 4-7; tT n-block jh
    # is the j-half.  Waves are ordered so the output DMA stream starts as
    # early as possible and never starves.
    # Full-j output units need both tT halves, so both x/tT halves come
    # first; blT + its weight transposes are still split by l-half so the
    # first units (needing only f-tiles 0-11) ship while f-tiles 12-23 are
    # still being produced.  biwT(0-12) sits between the tT halves to match
    # the input-DMA arrival order (x half 0, bi_w half A, x half 1, bi_w B).
    if _BLT_FULL:
        # full-s blT per l-half: waves then cover all 8 i-tiles of that
        # l-half at once (wave lh0 needs only f-tiles 0-11).  tT stays
        # split by s-half: a full-s tT would gate on x half 1 and open a
        # PE gap at startup that outweighs its 12 saved stationary loads
        # (sim: 129.0us vs 127.8us).
        emit_fcwT()
        emit_xT(0)
        emit_tT(0)
        emit_biwT(range(0, 12))
        emit_xT(1)
        emit_tT(1)
        emit_blT_full(range(0, 12))
        emit_wave((0, 1, 2, 3, 4, 5, 6, 7), lhs=(0,))
        emit_biwT(range(12, 24))
        emit_blT_full(range(12, 24))
        emit_wave((0, 1, 2, 3, 4, 5, 6, 7), lhs=(1,))
        return
    emit_fcwT()
    emit_xT(0)
    emit_tT(0)
    emit_biwT(range(0, 12))
    emit_xT(1)
    emit_tT(1)
    emit_blT(0, 512, range(0, 12))
    emit_wave((0, 1, 2, 3), lhs=(0,))
    emit_biwT(range(12, 24))
    emit_blT(0, 512, range(12, 24))
    emit_wave((0, 1, 2, 3), lhs=(1,))
    emit_blT(512, 512)
    emit_wave((4, 5, 6, 7))


def build_nc(unroll: int = 1):
    """Build the Bass program.  unroll>1 repeats the whole body (for timing
    measurements via wall-clock differencing)."""
    nc = bass.Bass(trn_type="TRN2")
    x_d = nc.dram_tensor("x", [S, IN], FP32, kind="ExternalInput")
    fcw_d = nc.dram_tensor("fc_w", [E, IN], FP32, kind="ExternalInput")
    fcb_d = nc.dram_tensor("fc_b", [E, 1], FP32, kind="ExternalInput")
    biw_d = nc.dram_tensor("bi_w", [E * L, E], FP32, kind="ExternalInput")
    bias_d = nc.dram_tensor("bias", [E, 1], FP32, kind="ExternalInput")
    ident_d = nc.dram_tensor("ident", [128, 128], BF16, kind="ExternalInput")
    out_d = nc.dram_tensor("out", [S, L, S], FP16, kind="ExternalOutput")
    dram = (x_d, fcw_d, fcb_d, biw_d, bias_d, out_d)

    with tile.TileContext(nc) as tc:
        with (
            tc.tile_pool(name="const", bufs=1) as const_pool,
            tc.tile_pool(name="big", bufs=1) as big_pool,
            tc.tile_pool(name="inp", bufs=1) as in_pool,
            tc.tile_pool(name="psum_mm", bufs=4, space="PSUM") as psum_mm,
            tc.tile_pool(name="stg", bufs=3) as stg_pool,
            tc.tile_pool(name="dram", bufs=1, space="DRAM") as dram_pool,
        ):
            pools = (const_pool, big_pool, in_pool, psum_mm, stg_pool,
                     dram_pool)
            ctr = [0, 0, 0]
            consts = _emit_consts(nc, const_pool, ident_d)
            for _ in range(unroll):
                _emit_body(nc, tc, pools, dram, ctr, consts)

    blob = _fix_sync_waits(nc.to_json_bytes())
    nc.to_json_bytes = lambda: blob
    return nc


_CACHE = {}


def _get_nc(unroll: int = 1):
    key = (unroll, _GFORM)
    if key not in _CACHE:
        _CACHE[key] = build_nc(unroll)
    return _CACHE[key]


def kernel(input_tensor, fc_w, fc_b, bi_w, bias):
    global _GFORM
    input_tensor = np.ascontiguousarray(np.asarray(input_tensor, dtype=np.float32))
    fc_w = np.ascontiguousarray(np.asarray(fc_w, dtype=np.float32))
    fc_b = np.ascontiguousarray(np.asarray(fc_b, dtype=np.float32)).reshape(E, 1)
    bi_w = np.ascontiguousarray(np.asarray(bi_w, dtype=np.float32))
    bias = np.ascontiguousarray(np.asarray(bias, dtype=np.float32)).reshape(E, 1)
    assert input_tensor.shape == (B, S, IN)

    # Measured head-to-head, the blT schedule is ~5us/body faster on this
    # hardware than the reassociated G-form (kept above for reference), so
    # it is used unconditionally.  It also applies the bias generally.
    _GFORM = False
    nc = _get_nc()
    ident = np.eye(128, dtype=ml_dtypes.bfloat16)
    in_maps = [
        {"x": input_tensor[c], "fc_w": fc_w, "fc_b": fc_b, "bi_w": bi_w,
         "bias": bias, "ident": ident}
        for c in range(N_CORES)
    ]
    res = run_bass_kernel_spmd(nc, in_maps, core_ids=list(range(N_CORES)))
    out = np.stack([res.results[c]["out"] for c in range(N_CORES)], axis=0)
    return out.astype(np.float32)

